# revision 47
# baseline (speedup 1.0000x reference)
"""Bezier soft-disk renderer on 8 Trainium2 NeuronCores.

Strategy (data-parallel over paths + associative over-compositing):
  Each core gets 128 of the 1024 paths. Front-to-back compositing
    canvas <- canvas*(1-m_p) + c_p*m_p
  is an affine map per pixel, so a shard of 128 consecutive paths
  composes to  canvas_out = canvas_in * A_s + B_s  with
    A_s = prod_p (1-m_p)
    B_s = sum_p c_p m_p prod_{q>p} (1-m_q).
  On-device, per core (paths on the 128 SBUF partitions):
    d2   = (gx-cx)^2 + (gy-cy)^2            (DVE tensor_scalar + ACT Square)
    m0   = Sigmoid(-50*sqrt(d2) + 50*r)     (ACT, per-partition bias)
    lg   = Ln(1 - alpha*m0)                 (ACT, per-partition scale)
    SS_k = sum_{q>=k} lg_q                  (TensorE, triangular ones matmul)
    V    = Exp(SS)                          (ACT)  -> V_0 = A_s
    B_s  = D^T @ V + c_last @ ones          (TensorE; D_k = c_{k-1}-c_k, D_0=-c_0,
                                             4th output row = V_0 = A_s)
  Host work is only shard/gather + the 8-term affine combine.
"""

import sys

if "/opt/trn_rl_repo" not in sys.path:
    sys.path.insert(0, "/opt/trn_rl_repo")

import numpy as np
from contextlib import ExitStack

H = W = 224
NPX = H * W
N_PATHS = 1024
PPC = 128           # paths per core
NCORES = 8
NSEG = 4
NSAMP = 50
NT = NSAMP - 1      # 49 samples per segment
NPTS = NSEG * NT    # 196 samples per path
INV_SOFT = 50.0     # 1/SOFTNESS
CHUNKS = [38, 38, 37, 37, 37, 37]  # rows per chunk (sum = 224)
MAXCH_PX = max(CHUNKS) * W  # biggest chunk, sizes the work tiles
BLK = 512                   # matmul moving-dim block (one PSUM bank)
XBLK = 1024                 # exp grouping (two PSUM banks, one ACT read)

_compiled = {}
last_results = None


def _build_nc():
    import concourse.tile as tile
    from concourse import bacc, mybir
    from concourse.tile_rust import add_dep_helper

    f32 = mybir.dt.float32
    f32r = mybir.dt.float32r
    bf16 = mybir.dt.bfloat16
    f16 = mybir.dt.float16
    ACT = mybir.ActivationFunctionType
    ALU = mybir.AluOpType

    nc = bacc.Bacc("TRN2", target_bir_lowering=False, debug=False,
                   num_devices=NCORES)

    cp_d = nc.dram_tensor("cp", [PPC, NSEG * 4 * 2], f32, kind="ExternalInput").ap()
    col_d = nc.dram_tensor("col", [PPC, 4], f32, kind="ExternalInput").ap()
    lin_d = nc.dram_tensor("lin_t", [PPC, W], f32, kind="ExternalInput").ap()
    bas_d = nc.dram_tensor("basis", [PPC, 4 * NT], f32, kind="ExternalInput").ap()
    wng_d = nc.dram_tensor("wneg", [PPC, NSEG * 4 * 2], f32, kind="ExternalInput").ap()
    tri_d = nc.dram_tensor("tri", [PPC, PPC], f16, kind="ExternalInput").ap()
    ab_d = nc.dram_tensor("AB", [4, NPX], f32, kind="ExternalOutput").ap()

    with ExitStack() as ctx:
        tc = ctx.enter_context(tile.TileContext(nc))

        singles = ctx.enter_context(tc.tile_pool(name="singles", bufs=1))
        setup = ctx.enter_context(tc.tile_pool(name="setup", bufs=1))
        work = ctx.enter_context(tc.tile_pool(name="work", bufs=3))
        lgpool = ctx.enter_context(tc.tile_pool(name="lg", bufs=2))
        vpool = ctx.enter_context(tc.tile_pool(name="vp", bufs=4))
        bstage = ctx.enter_context(tc.tile_pool(name="bst", bufs=6))
        ps_ss = ctx.enter_context(tc.tile_pool(name="pss", bufs=3, space="PSUM"))
        ps_b = ctx.enter_context(tc.tile_pool(name="psb", bufs=2, space="PSUM"))

        # ---- load inputs -------------------------------------------------
        cp_sb = singles.tile([PPC, NSEG * 4 * 2], f32)
        nc.sync.dma_start(cp_sb[:], cp_d)
        col_sb = singles.tile([PPC, 4], f32)
        nc.sync.dma_start(col_sb[:], col_d)
        lin_sb = singles.tile([PPC, W], f32)
        nc.sync.dma_start(lin_sb[:], lin_d)
        bas_sb = singles.tile([PPC, 4 * NT], f32)
        nc.sync.dma_start(bas_sb[:], bas_d)
        wng_sb = singles.tile([PPC, NSEG * 4 * 2], f32)
        nc.sync.dma_start(wng_sb[:], wng_d)
        tri_sb = singles.tile([PPC, PPC], f16)
        nc.sync.dma_start(tri_sb[:], tri_d)

        # ---- centers (negated means): a fixed 16-weight functional of the
        # control points -- ready ~3us before the full pts chain ------------
        cprod = setup.tile([PPC, NSEG * 4 * 2], f32)
        nc.vector.tensor_mul(cprod[:], cp_sb[:], wng_sb[:])
        cp3 = cprod[:].rearrange("p (s c e) -> p s c e", s=NSEG, c=4, e=2)
        neg_cx = setup.tile([PPC, 1], f32)
        nc.vector.tensor_reduce(neg_cx[:], cp3[:, :, :, 0],
                                axis=mybir.AxisListType.XY, op=ALU.add)
        neg_cy = setup.tile([PPC, 1], f32)
        nc.vector.tensor_reduce(neg_cy[:], cp3[:, :, :, 1],
                                axis=mybir.AxisListType.XY, op=ALU.add)

        # ---- bezier samples: pts[p,s,t,e] = sum_c basis[t,c]*cp[p,s,c,e] -
        cp4 = cp_sb[:].rearrange("p (s c e) -> p s c e", s=NSEG, c=4, e=2)
        bas4 = bas_sb[:].rearrange("p (c t) -> p c t", c=4, t=NT)
        prods = []
        for c in range(4):
            pc = setup.tile([PPC, NSEG, NT, 2], f32, tag=f"prod{c}")
            cpv = cp4[:, :, c, :].unsqueeze(2).broadcast_to([PPC, NSEG, NT, 2])
            bv = (bas4[:, c, :].unsqueeze(1).unsqueeze(3)
                  .broadcast_to([PPC, NSEG, NT, 2]))
            nc.vector.tensor_mul(pc[:], cpv, bv)
            prods.append(pc)
        s01 = setup.tile([PPC, NSEG, NT, 2], f32)
        nc.vector.tensor_add(s01[:], prods[0][:], prods[1][:])
        s23 = setup.tile([PPC, NSEG, NT, 2], f32)
        nc.vector.tensor_add(s23[:], prods[2][:], prods[3][:])
        pts = setup.tile([PPC, NSEG, NT, 2], f32)
        nc.vector.tensor_add(pts[:], s01[:], s23[:])

        ptx = pts[:, :, :, 0]   # [p, 4, 49]
        pty = pts[:, :, :, 1]

        # ---- avg radius -> r50 = 50 * mean ||pts - c|| -------------------
        sqx = setup.tile([PPC, NSEG, NT], f32)
        nc.scalar.activation(sqx[:], ptx, ACT.Square, bias=neg_cx[:])
        sqy = setup.tile([PPC, NSEG, NT], f32)
        nc.scalar.activation(sqy[:], pty, ACT.Square, bias=neg_cy[:])
        d2p = setup.tile([PPC, NSEG, NT], f32)
        nc.vector.tensor_add(d2p[:], sqx[:], sqy[:])
        sp = setup.tile([PPC, NSEG, NT], f32)
        rsum = setup.tile([PPC, 1], f32)
        nc.scalar.activation(sp[:], d2p[:], ACT.Sqrt, accum_out=rsum[:])
        r50 = setup.tile([PPC, 1], f32)
        nc.vector.tensor_scalar_mul(r50[:], rsum[:], INV_SOFT / NPTS)

        # ---- per-path alpha and color-diff matmul weights ----------------
        neg_alpha = setup.tile([PPC, 1], f32)
        nc.vector.tensor_scalar_mul(neg_alpha[:], col_sb[:, 3:4], -1.0)

        csh = setup.tile([PPC, 3], f32)       # c_{k-1} (0 for k=0)
        nc.vector.memset(csh[0:1, :], 0.0)
        nc.sync.dma_start(csh[1:PPC, :], col_sb[0:PPC - 1, 0:3])
        d4f = setup.tile([PPC, 4], f32)       # cols 0-2: D, col 3: e_0 (A row)
        nc.vector.tensor_sub(d4f[:, 0:3], csh[:], col_sb[:, 0:3])
        nc.vector.memset(d4f[:, 3:4], 0.0)
        nc.vector.memset(d4f[0:1, 3:4], 1.0)
        d4 = setup.tile([PPC, 4], f16)
        nc.vector.tensor_copy(d4[:], d4f[:])



        # ---- separable squared distances --------------------------------
        dx2 = singles.tile([PPC, W], f32)
        nc.scalar.activation(dx2[:], lin_sb[:], ACT.Square, bias=neg_cx[:])
        dy2 = singles.tile([PPC, W], f32)
        nc.scalar.activation(dy2[:], lin_sb[:], ACT.Square, bias=neg_cy[:])

        # ---- main loop: pairs of chunks, phase-batched to cut ACT table
        # switches (sqrt x2 | sigmoid x2 | ln x2 | exp xN per pair).
        # work pool has 3 buffers and the next pair's d2 adds are emitted
        # right after this pair's ln phase, so the DVE finishes them long
        # before the next sqrt phase (no pair-boundary stall).
        prev_act = None  # last ACT inst of previous phase, for ordering
        npairs = len(CHUNKS) // 2
        pair_rows = [(sum(CHUNKS[:2 * p]), CHUNKS[2 * p], CHUNKS[2 * p + 1])
                     for p in range(npairs)]
        tts_by_pair = {}

        def ordered_act(i):
            nonlocal prev_act
            if prev_act is not None:
                add_dep_helper(i.ins, prev_act.ins, sync=False,
                               reason="ACT table-set phase order")
            prev_act = i

        def emit_d2(p, k, prime=False):
            """DVE broadcast add d2 = dy2[rows] + dx2 for chunk k of pair p."""
            r0 = pair_rows[p][0] + (pair_rows[p][1] if k else 0)
            ch_rows = pair_rows[p][1 + k]
            t = work.tile([PPC, MAXCH_PX], f32, tag="work")
            t3 = t[:, :ch_rows * W].rearrange("p (r j) -> p r j",
                                              r=ch_rows, j=W)
            nsub = 4 if prime else 1
            rstep = -(-ch_rows // nsub)
            for rlo in range(0, ch_rows, rstep):
                rn = min(rstep, ch_rows - rlo)
                dyb = (dy2[:, r0 + rlo:r0 + rlo + rn].unsqueeze(2)
                       .broadcast_to([PPC, rn, W]))
                dxb = dx2[:].unsqueeze(1).broadcast_to([PPC, rn, W])
                nc.vector.tensor_add(t3[:, rlo:rlo + rn, :], dyb, dxb)
            tts_by_pair.setdefault(p, []).append(t)

        emit_d2(0, 0, prime=True)
        emit_d2(0, 1)

        for pair in range(npairs):
            row0, rows_a, rows_b = pair_rows[pair]
            tts = tts_by_pair.pop(pair)
            metas = [(row0, rows_a * W), (row0 + rows_a, rows_b * W)]

            for k in range(2):
                ch_px = metas[k][1]
                if pair == 0:
                    # prime the pipeline: sub-slice the first sqrt so ACT
                    # starts before the full d2 add has finished
                    step = -(-ch_px // 4)
                    for lo in range(0, ch_px, step):
                        n = min(step, ch_px - lo)
                        ordered_act(nc.scalar.activation(
                            tts[k][:, lo:lo + n], tts[k][:, lo:lo + n],
                            ACT.Sqrt))
                else:
                    ordered_act(nc.scalar.activation(
                        tts[k][:, :ch_px], tts[k][:, :ch_px], ACT.Sqrt))
            for k in range(2):
                ch_px = metas[k][1]
                ordered_act(nc.scalar.activation(
                    tts[k][:, :ch_px], tts[k][:, :ch_px],
                    ACT.Sigmoid, bias=r50[:], scale=-INV_SOFT))
            lgs = []
            for k in range(2):
                ch_px = metas[k][1]
                lg = lgpool.tile([PPC, MAXCH_PX], f16, tag="lg")
                ordered_act(nc.scalar.activation(
                    lg[:, :ch_px], tts[k][:, :ch_px],
                    ACT.Ln, bias=1.0, scale=neg_alpha[:]))
                lgs.append(lg)

            # queue the next pair's d2 now: one work buffer is already free
            # and the second frees after this pair's first ln
            if pair + 1 < npairs:
                emit_d2(pair + 1, 0)
                emit_d2(pair + 1, 1)

            for k in range(2):
                crow0, ch_px = metas[k]
                lg = lgs[k]
                ngrp = (ch_px + XBLK - 1) // XBLK
                for g in range(ngrp):
                    lo = g * XBLK
                    gw = min(XBLK, ch_px - lo)
                    px0 = crow0 * W + lo

                    ss = ps_ss.tile([PPC, XBLK], f32, tag="ss")
                    for h in range(0, gw, BLK):
                        hw_ = min(BLK, gw - h)
                        nc.tensor.matmul(ss[:, h:h + hw_], tri_sb[:],
                                         lg[:, lo + h:lo + h + hw_],
                                         start=True, stop=True)
                    v = vpool.tile([PPC, XBLK], f16, tag="v")
                    ordered_act(nc.scalar.activation(v[:, :gw], ss[:, :gw],
                                                     ACT.Exp))

                    for h in range(0, gw, BLK):
                        hw_ = min(BLK, gw - h)
                        bp = ps_b.tile([4, BLK], f32, tag="bp")
                        nc.tensor.matmul(bp[:, :hw_], d4[:],
                                         v[:, h:h + hw_],
                                         start=True, stop=True)
                        bs = bstage.tile([4, BLK], f32, tag="bs")
                        nc.vector.tensor_copy(bs[:, :hw_], bp[:, :hw_])
                        nc.sync.dma_start(ab_d[:, px0 + h:px0 + h + hw_],
                                          bs[:, :hw_])

    nc.compile()
    return nc


def _get_nc():
    if "nc" not in _compiled:
        _compiled["nc"] = _build_nc()
    return _compiled["nc"]


def _bezier_basis():
    t = np.linspace(0.0, 1.0, NSAMP, dtype=np.float32)[:-1]
    mt = 1.0 - t
    return np.stack([mt ** 3, 3.0 * mt ** 2 * t, 3.0 * mt * t ** 2, t ** 3],
                    axis=-1).astype(np.float32)  # (49, 4)


def _run_on_device(cp, col):
    """Compile (cached) + run the SPMD kernel; returns list of AB arrays."""
    global last_results
    from concourse.bass_utils import run_bass_kernel_spmd

    basis = _bezier_basis()                       # (49, 4)
    bas_in = np.broadcast_to(basis.T.reshape(1, 4 * NT),
                             (PPC, 4 * NT)).copy()  # rows: c-major
    lin = np.linspace(0.0, 1.0, W, dtype=np.float32)
    lin_in = np.broadcast_to(lin, (PPC, W)).copy()
    u = -(basis.sum(axis=0) / NPTS).astype(np.float32)       # (4,) per c
    wneg = np.broadcast_to(np.repeat(u, 2)[None, None, :],
                           (PPC, NSEG, 8)).reshape(PPC, 32).copy()
    q = np.arange(PPC)
    tri = (q[:, None] >= q[None, :]).astype(np.float16)  # tri[q,k] = q>=k

    nc = _get_nc()
    in_maps = []
    for s in range(NCORES):
        sl = slice(s * PPC, (s + 1) * PPC)
        in_maps.append({
            "cp": cp[sl].reshape(PPC, NSEG * 4 * 2).copy(),
            "col": col[sl].copy(),
            "lin_t": lin_in,
            "basis": bas_in,
            "wneg": wneg,
            "tri": tri,
        })

    res = run_bass_kernel_spmd(nc, in_maps, core_ids=list(range(NCORES)))
    last_results = res
    return [res.results[s]["AB"] for s in range(NCORES)]


def _subproc_main(in_path, out_path):
    data = np.load(in_path)
    abs_ = _run_on_device(data["cp"], data["col"])
    np.savez(out_path, **{f"ab{s}": ab for s, ab in enumerate(abs_)})


def _run_with_recovery(cp, col):
    """The NeuronCore runtime occasionally reports a transient
    NRT_EXEC_UNIT_UNRECOVERABLE on a cold run, which poisons the whole PJRT
    client in this process. A fresh process reliably recovers, so fall back
    to re-running in a subprocess."""
    import os
    import subprocess
    import sys as _sys
    import tempfile

    try:
        return _run_on_device(cp, col)
    except Exception:
        pass
    last_exc = None
    for _ in range(3):
        tmp = tempfile.mkdtemp()
        in_path = os.path.join(tmp, "in.npz")
        out_path = os.path.join(tmp, "out.npz")
        np.savez(in_path, cp=cp, col=col)
        code = ("import sys; sys.path.insert(0, %r); import kernel; "
                "kernel._subproc_main(%r, %r)"
                % (os.path.dirname(os.path.abspath(__file__)),
                   in_path, out_path))
        try:
            subprocess.run([_sys.executable, "-c", code], check=True,
                           timeout=1200)
            data = np.load(out_path)
            return [data[f"ab{s}"] for s in range(NCORES)]
        except Exception as e:  # noqa: PERF203
            last_exc = e
    raise last_exc


def kernel(paths_control_points, colors):
    cp = np.ascontiguousarray(paths_control_points, dtype=np.float32)
    col = np.ascontiguousarray(colors, dtype=np.float32)

    abs_ = _run_with_recovery(cp, col)

    canvas = np.ones((3, H, W), dtype=np.float32)
    for s in range(NCORES):
        ab = abs_[s]
        a = ab[3].reshape(H, W)
        b = ab[0:3].reshape(3, H, W)
        c_last = col[s * PPC + PPC - 1, 0:3]
        canvas = canvas * a[None] + b + c_last[:, None, None]
    return canvas.astype(np.float32)


# revision 48
# speedup vs baseline: 1.0097x; 1.0097x over previous
"""Bezier soft-disk renderer on 8 Trainium2 NeuronCores.

Strategy (data-parallel over paths + associative over-compositing):
  Each core gets 128 of the 1024 paths. Front-to-back compositing
    canvas <- canvas*(1-m_p) + c_p*m_p
  is an affine map per pixel, so a shard of 128 consecutive paths
  composes to  canvas_out = canvas_in * A_s + B_s  with
    A_s = prod_p (1-m_p)
    B_s = sum_p c_p m_p prod_{q>p} (1-m_q).
  On-device, per core (paths on the 128 SBUF partitions):
    d2   = (gx-cx)^2 + (gy-cy)^2            (DVE tensor_scalar + ACT Square)
    m0   = Sigmoid(-50*sqrt(d2) + 50*r)     (ACT, per-partition bias)
    lg   = Ln(1 - alpha*m0)                 (ACT, per-partition scale)
    SS_k = sum_{q>=k} lg_q                  (TensorE, triangular ones matmul)
    V    = Exp(SS)                          (ACT)  -> V_0 = A_s
    B_s  = D^T @ V + c_last @ ones          (TensorE; D_k = c_{k-1}-c_k, D_0=-c_0,
                                             4th output row = V_0 = A_s)
  Host work is only shard/gather + the 8-term affine combine.
"""

import sys

if "/opt/trn_rl_repo" not in sys.path:
    sys.path.insert(0, "/opt/trn_rl_repo")

import numpy as np
from contextlib import ExitStack

H = W = 224
NPX = H * W
N_PATHS = 1024
PPC = 128           # paths per core
NCORES = 8
NSEG = 4
NSAMP = 50
NT = NSAMP - 1      # 49 samples per segment
NPTS = NSEG * NT    # 196 samples per path
INV_SOFT = 50.0     # 1/SOFTNESS
CHUNKS = [38, 38, 37, 37, 37, 37]  # rows per chunk (sum = 224)
MAXCH_PX = max(CHUNKS) * W  # biggest chunk, sizes the work tiles
BLK = 512                   # matmul moving-dim block (one PSUM bank)
XBLK = 512                  # suffix-matmul/exp grouping (one PSUM bank)

_compiled = {}
last_results = None


def _build_nc():
    import concourse.tile as tile
    from concourse import bacc, mybir
    from concourse.tile_rust import add_dep_helper

    f32 = mybir.dt.float32
    f32r = mybir.dt.float32r
    bf16 = mybir.dt.bfloat16
    f16 = mybir.dt.float16
    ACT = mybir.ActivationFunctionType
    ALU = mybir.AluOpType

    nc = bacc.Bacc("TRN2", target_bir_lowering=False, debug=False,
                   num_devices=NCORES)

    cp_d = nc.dram_tensor("cp", [PPC, NSEG * 4 * 2], f32, kind="ExternalInput").ap()
    col_d = nc.dram_tensor("col", [PPC, 4], f32, kind="ExternalInput").ap()
    lin_d = nc.dram_tensor("lin_t", [PPC, W], f32, kind="ExternalInput").ap()
    bas_d = nc.dram_tensor("basis", [PPC, 4 * NT], f32, kind="ExternalInput").ap()
    wng_d = nc.dram_tensor("wneg", [PPC, NSEG * 4 * 2], f32, kind="ExternalInput").ap()
    tri_d = nc.dram_tensor("tri", [PPC, PPC], f16, kind="ExternalInput").ap()
    ab_d = nc.dram_tensor("AB", [4, NPX], f32, kind="ExternalOutput").ap()

    with ExitStack() as ctx:
        tc = ctx.enter_context(tile.TileContext(nc))

        singles = ctx.enter_context(tc.tile_pool(name="singles", bufs=1))
        setup = ctx.enter_context(tc.tile_pool(name="setup", bufs=1))
        work = ctx.enter_context(tc.tile_pool(name="work", bufs=3))
        lgpool = ctx.enter_context(tc.tile_pool(name="lg", bufs=2))
        vpool = ctx.enter_context(tc.tile_pool(name="vp", bufs=8))
        bstage = ctx.enter_context(tc.tile_pool(name="bst", bufs=6))
        ps_ss = ctx.enter_context(tc.tile_pool(name="pss", bufs=6, space="PSUM"))
        ps_b = ctx.enter_context(tc.tile_pool(name="psb", bufs=2, space="PSUM"))

        # ---- load inputs -------------------------------------------------
        cp_sb = singles.tile([PPC, NSEG * 4 * 2], f32)
        nc.sync.dma_start(cp_sb[:], cp_d)
        col_sb = singles.tile([PPC, 4], f32)
        nc.sync.dma_start(col_sb[:], col_d)
        lin_sb = singles.tile([PPC, W], f32)
        nc.sync.dma_start(lin_sb[:], lin_d)
        bas_sb = singles.tile([PPC, 4 * NT], f32)
        nc.sync.dma_start(bas_sb[:], bas_d)
        wng_sb = singles.tile([PPC, NSEG * 4 * 2], f32)
        nc.sync.dma_start(wng_sb[:], wng_d)
        tri_sb = singles.tile([PPC, PPC], f16)
        nc.sync.dma_start(tri_sb[:], tri_d)

        # ---- centers (negated means): a fixed 16-weight functional of the
        # control points -- ready ~3us before the full pts chain ------------
        cprod = setup.tile([PPC, NSEG * 4 * 2], f32)
        nc.vector.tensor_mul(cprod[:], cp_sb[:], wng_sb[:])
        cp3 = cprod[:].rearrange("p (s c e) -> p s c e", s=NSEG, c=4, e=2)
        neg_cx = setup.tile([PPC, 1], f32)
        nc.vector.tensor_reduce(neg_cx[:], cp3[:, :, :, 0],
                                axis=mybir.AxisListType.XY, op=ALU.add)
        neg_cy = setup.tile([PPC, 1], f32)
        nc.vector.tensor_reduce(neg_cy[:], cp3[:, :, :, 1],
                                axis=mybir.AxisListType.XY, op=ALU.add)

        # ---- bezier samples: pts[p,s,t,e] = sum_c basis[t,c]*cp[p,s,c,e] -
        cp4 = cp_sb[:].rearrange("p (s c e) -> p s c e", s=NSEG, c=4, e=2)
        bas4 = bas_sb[:].rearrange("p (c t) -> p c t", c=4, t=NT)
        prods = []
        for c in range(4):
            pc = setup.tile([PPC, NSEG, NT, 2], f32, tag=f"prod{c}")
            cpv = cp4[:, :, c, :].unsqueeze(2).broadcast_to([PPC, NSEG, NT, 2])
            bv = (bas4[:, c, :].unsqueeze(1).unsqueeze(3)
                  .broadcast_to([PPC, NSEG, NT, 2]))
            nc.vector.tensor_mul(pc[:], cpv, bv)
            prods.append(pc)
        s01 = setup.tile([PPC, NSEG, NT, 2], f32)
        nc.vector.tensor_add(s01[:], prods[0][:], prods[1][:])
        s23 = setup.tile([PPC, NSEG, NT, 2], f32)
        nc.vector.tensor_add(s23[:], prods[2][:], prods[3][:])
        pts = setup.tile([PPC, NSEG, NT, 2], f32)
        nc.vector.tensor_add(pts[:], s01[:], s23[:])

        ptx = pts[:, :, :, 0]   # [p, 4, 49]
        pty = pts[:, :, :, 1]

        # ---- avg radius -> r50 = 50 * mean ||pts - c|| -------------------
        sqx = setup.tile([PPC, NSEG, NT], f32)
        nc.scalar.activation(sqx[:], ptx, ACT.Square, bias=neg_cx[:])
        sqy = setup.tile([PPC, NSEG, NT], f32)
        nc.scalar.activation(sqy[:], pty, ACT.Square, bias=neg_cy[:])
        d2p = setup.tile([PPC, NSEG, NT], f32)
        nc.vector.tensor_add(d2p[:], sqx[:], sqy[:])
        sp = setup.tile([PPC, NSEG, NT], f32)
        rsum = setup.tile([PPC, 1], f32)
        nc.scalar.activation(sp[:], d2p[:], ACT.Sqrt, accum_out=rsum[:])
        r50 = setup.tile([PPC, 1], f32)
        nc.vector.tensor_scalar_mul(r50[:], rsum[:], INV_SOFT / NPTS)

        # ---- per-path alpha and color-diff matmul weights ----------------
        neg_alpha = setup.tile([PPC, 1], f32)
        nc.vector.tensor_scalar_mul(neg_alpha[:], col_sb[:, 3:4], -1.0)

        csh = setup.tile([PPC, 3], f32)       # c_{k-1} (0 for k=0)
        nc.vector.memset(csh[0:1, :], 0.0)
        nc.sync.dma_start(csh[1:PPC, :], col_sb[0:PPC - 1, 0:3])
        d4f = setup.tile([PPC, 4], f32)       # cols 0-2: D, col 3: e_0 (A row)
        nc.vector.tensor_sub(d4f[:, 0:3], csh[:], col_sb[:, 0:3])
        nc.vector.memset(d4f[:, 3:4], 0.0)
        nc.vector.memset(d4f[0:1, 3:4], 1.0)
        d4 = setup.tile([PPC, 4], f16)
        nc.vector.tensor_copy(d4[:], d4f[:])



        # ---- separable squared distances --------------------------------
        dx2 = singles.tile([PPC, W], f32)
        nc.scalar.activation(dx2[:], lin_sb[:], ACT.Square, bias=neg_cx[:])
        dy2 = singles.tile([PPC, W], f32)
        nc.scalar.activation(dy2[:], lin_sb[:], ACT.Square, bias=neg_cy[:])

        # ---- main loop: pairs of chunks, phase-batched to cut ACT table
        # switches (sqrt x2 | sigmoid x2 | ln x2 | exp xN per pair).
        # work pool has 3 buffers and the next pair's d2 adds are emitted
        # right after this pair's ln phase, so the DVE finishes them long
        # before the next sqrt phase (no pair-boundary stall).
        prev_act = None  # last ACT inst of previous phase, for ordering
        npairs = len(CHUNKS) // 2
        pair_rows = [(sum(CHUNKS[:2 * p]), CHUNKS[2 * p], CHUNKS[2 * p + 1])
                     for p in range(npairs)]
        tts_by_pair = {}

        def ordered_act(i):
            nonlocal prev_act
            if prev_act is not None:
                add_dep_helper(i.ins, prev_act.ins, sync=False,
                               reason="ACT table-set phase order")
            prev_act = i

        def emit_d2(p, k, prime=False):
            """DVE broadcast add d2 = dy2[rows] + dx2 for chunk k of pair p."""
            r0 = pair_rows[p][0] + (pair_rows[p][1] if k else 0)
            ch_rows = pair_rows[p][1 + k]
            t = work.tile([PPC, MAXCH_PX], f32, tag="work")
            t3 = t[:, :ch_rows * W].rearrange("p (r j) -> p r j",
                                              r=ch_rows, j=W)
            nsub = 4 if prime else 1
            rstep = -(-ch_rows // nsub)
            for rlo in range(0, ch_rows, rstep):
                rn = min(rstep, ch_rows - rlo)
                dyb = (dy2[:, r0 + rlo:r0 + rlo + rn].unsqueeze(2)
                       .broadcast_to([PPC, rn, W]))
                dxb = dx2[:].unsqueeze(1).broadcast_to([PPC, rn, W])
                nc.vector.tensor_add(t3[:, rlo:rlo + rn, :], dyb, dxb)
            tts_by_pair.setdefault(p, []).append(t)

        emit_d2(0, 0, prime=True)
        emit_d2(0, 1)

        for pair in range(npairs):
            row0, rows_a, rows_b = pair_rows[pair]
            tts = tts_by_pair.pop(pair)
            metas = [(row0, rows_a * W), (row0 + rows_a, rows_b * W)]

            for k in range(2):
                ch_px = metas[k][1]
                if pair == 0:
                    # prime the pipeline: sub-slice the first sqrt so ACT
                    # starts before the full d2 add has finished
                    step = -(-ch_px // 4)
                    for lo in range(0, ch_px, step):
                        n = min(step, ch_px - lo)
                        ordered_act(nc.scalar.activation(
                            tts[k][:, lo:lo + n], tts[k][:, lo:lo + n],
                            ACT.Sqrt))
                else:
                    ordered_act(nc.scalar.activation(
                        tts[k][:, :ch_px], tts[k][:, :ch_px], ACT.Sqrt))
            for k in range(2):
                ch_px = metas[k][1]
                ordered_act(nc.scalar.activation(
                    tts[k][:, :ch_px], tts[k][:, :ch_px],
                    ACT.Sigmoid, bias=r50[:], scale=-INV_SOFT))
            lgs = []
            for k in range(2):
                ch_px = metas[k][1]
                lg = lgpool.tile([PPC, MAXCH_PX], f16, tag="lg")
                ordered_act(nc.scalar.activation(
                    lg[:, :ch_px], tts[k][:, :ch_px],
                    ACT.Ln, bias=1.0, scale=neg_alpha[:]))
                lgs.append(lg)

            # queue the next pair's d2 now: one work buffer is already free
            # and the second frees after this pair's first ln
            if pair + 1 < npairs:
                emit_d2(pair + 1, 0)
                emit_d2(pair + 1, 1)

            for k in range(2):
                crow0, ch_px = metas[k]
                lg = lgs[k]
                ngrp = (ch_px + XBLK - 1) // XBLK
                for g in range(ngrp):
                    lo = g * XBLK
                    gw = min(XBLK, ch_px - lo)
                    px0 = crow0 * W + lo

                    ss = ps_ss.tile([PPC, XBLK], f32, tag="ss")
                    nc.tensor.matmul(ss[:, :gw], tri_sb[:],
                                     lg[:, lo:lo + gw],
                                     start=True, stop=True)
                    v = vpool.tile([PPC, XBLK], f16, tag="v")
                    ordered_act(nc.scalar.activation(v[:, :gw], ss[:, :gw],
                                                     ACT.Exp))

                    bp = ps_b.tile([4, XBLK], f32, tag="bp")
                    nc.tensor.matmul(bp[:, :gw], d4[:], v[:, :gw],
                                     start=True, stop=True)
                    bs = bstage.tile([4, XBLK], f32, tag="bs")
                    nc.vector.tensor_copy(bs[:, :gw], bp[:, :gw])
                    nc.sync.dma_start(ab_d[:, px0:px0 + gw], bs[:, :gw])

    nc.compile()
    return nc


def _get_nc():
    if "nc" not in _compiled:
        _compiled["nc"] = _build_nc()
    return _compiled["nc"]


def _bezier_basis():
    t = np.linspace(0.0, 1.0, NSAMP, dtype=np.float32)[:-1]
    mt = 1.0 - t
    return np.stack([mt ** 3, 3.0 * mt ** 2 * t, 3.0 * mt * t ** 2, t ** 3],
                    axis=-1).astype(np.float32)  # (49, 4)


def _run_on_device(cp, col):
    """Compile (cached) + run the SPMD kernel; returns list of AB arrays."""
    global last_results
    from concourse.bass_utils import run_bass_kernel_spmd

    basis = _bezier_basis()                       # (49, 4)
    bas_in = np.broadcast_to(basis.T.reshape(1, 4 * NT),
                             (PPC, 4 * NT)).copy()  # rows: c-major
    lin = np.linspace(0.0, 1.0, W, dtype=np.float32)
    lin_in = np.broadcast_to(lin, (PPC, W)).copy()
    u = -(basis.sum(axis=0) / NPTS).astype(np.float32)       # (4,) per c
    wneg = np.broadcast_to(np.repeat(u, 2)[None, None, :],
                           (PPC, NSEG, 8)).reshape(PPC, 32).copy()
    q = np.arange(PPC)
    tri = (q[:, None] >= q[None, :]).astype(np.float16)  # tri[q,k] = q>=k

    nc = _get_nc()
    in_maps = []
    for s in range(NCORES):
        sl = slice(s * PPC, (s + 1) * PPC)
        in_maps.append({
            "cp": cp[sl].reshape(PPC, NSEG * 4 * 2).copy(),
            "col": col[sl].copy(),
            "lin_t": lin_in,
            "basis": bas_in,
            "wneg": wneg,
            "tri": tri,
        })

    res = run_bass_kernel_spmd(nc, in_maps, core_ids=list(range(NCORES)))
    last_results = res
    return [res.results[s]["AB"] for s in range(NCORES)]


def _subproc_main(in_path, out_path):
    data = np.load(in_path)
    abs_ = _run_on_device(data["cp"], data["col"])
    np.savez(out_path, **{f"ab{s}": ab for s, ab in enumerate(abs_)})


def _run_with_recovery(cp, col):
    """The NeuronCore runtime occasionally reports a transient
    NRT_EXEC_UNIT_UNRECOVERABLE on a cold run, which poisons the whole PJRT
    client in this process. A fresh process reliably recovers, so fall back
    to re-running in a subprocess."""
    import os
    import subprocess
    import sys as _sys
    import tempfile

    try:
        return _run_on_device(cp, col)
    except Exception:
        pass
    last_exc = None
    for _ in range(3):
        tmp = tempfile.mkdtemp()
        in_path = os.path.join(tmp, "in.npz")
        out_path = os.path.join(tmp, "out.npz")
        np.savez(in_path, cp=cp, col=col)
        code = ("import sys; sys.path.insert(0, %r); import kernel; "
                "kernel._subproc_main(%r, %r)"
                % (os.path.dirname(os.path.abspath(__file__)),
                   in_path, out_path))
        try:
            subprocess.run([_sys.executable, "-c", code], check=True,
                           timeout=1200)
            data = np.load(out_path)
            return [data[f"ab{s}"] for s in range(NCORES)]
        except Exception as e:  # noqa: PERF203
            last_exc = e
    raise last_exc


def kernel(paths_control_points, colors):
    cp = np.ascontiguousarray(paths_control_points, dtype=np.float32)
    col = np.ascontiguousarray(colors, dtype=np.float32)

    abs_ = _run_with_recovery(cp, col)

    canvas = np.ones((3, H, W), dtype=np.float32)
    for s in range(NCORES):
        ab = abs_[s]
        a = ab[3].reshape(H, W)
        b = ab[0:3].reshape(3, H, W)
        c_last = col[s * PPC + PPC - 1, 0:3]
        canvas = canvas * a[None] + b + c_last[:, None, None]
    return canvas.astype(np.float32)


# revision 51
# speedup vs baseline: 1.0235x; 1.0137x over previous
"""Bezier soft-disk renderer on 8 Trainium2 NeuronCores.

Strategy (data-parallel over paths + associative over-compositing):
  Each core gets 128 of the 1024 paths. Front-to-back compositing
    canvas <- canvas*(1-m_p) + c_p*m_p
  is an affine map per pixel, so a shard of 128 consecutive paths
  composes to  canvas_out = canvas_in * A_s + B_s  with
    A_s = prod_p (1-m_p)
    B_s = sum_p c_p m_p prod_{q>p} (1-m_q).
  On-device, per core (paths on the 128 SBUF partitions):
    d2   = (gx-cx)^2 + (gy-cy)^2            (DVE tensor_scalar + ACT Square)
    m0   = Sigmoid(-50*sqrt(d2) + 50*r)     (ACT, per-partition bias)
    lg   = Ln(1 - alpha*m0)                 (ACT, per-partition scale)
    SS_k = sum_{q>=k} lg_q                  (TensorE, triangular ones matmul)
    V    = Exp(SS)                          (ACT)  -> V_0 = A_s
    B_s  = D^T @ V + c_last @ ones          (TensorE; D_k = c_{k-1}-c_k, D_0=-c_0,
                                             4th output row = V_0 = A_s)
  Host work is only shard/gather + the 8-term affine combine.
"""

import sys

if "/opt/trn_rl_repo" not in sys.path:
    sys.path.insert(0, "/opt/trn_rl_repo")

import numpy as np
from contextlib import ExitStack

H = W = 224
NPX = H * W
N_PATHS = 1024
PPC = 128           # paths per core
NCORES = 8
NSEG = 4
NSAMP = 50
NT = NSAMP - 1      # 49 samples per segment
NPTS = NSEG * NT    # 196 samples per path
INV_SOFT = 50.0     # 1/SOFTNESS
CHUNKS = [38, 38, 37, 37, 37, 37]  # rows per chunk (sum = 224)
MAXCH_PX = max(CHUNKS) * W  # biggest chunk, sizes the work tiles
BLK = 512                   # matmul moving-dim block (one PSUM bank)
XBLK = 512                  # suffix-matmul/exp grouping (one PSUM bank)

_compiled = {}
last_results = None


def _build_nc():
    import concourse.tile as tile
    from concourse import bacc, mybir
    from concourse.tile_rust import add_dep_helper

    f32 = mybir.dt.float32
    f32r = mybir.dt.float32r
    bf16 = mybir.dt.bfloat16
    f16 = mybir.dt.float16
    ACT = mybir.ActivationFunctionType
    ALU = mybir.AluOpType

    nc = bacc.Bacc("TRN2", target_bir_lowering=False, debug=False,
                   num_devices=NCORES)

    cp_d = nc.dram_tensor("cp", [PPC, NSEG * 4 * 2], f32, kind="ExternalInput").ap()
    col_d = nc.dram_tensor("col", [PPC, 4], f32, kind="ExternalInput").ap()
    lin_d = nc.dram_tensor("lin_t", [PPC, W], f32, kind="ExternalInput").ap()
    bas_d = nc.dram_tensor("basis", [PPC, 4 * NT], f32, kind="ExternalInput").ap()
    wng_d = nc.dram_tensor("wneg", [PPC, NSEG * 4 * 2], f32, kind="ExternalInput").ap()
    tri_d = nc.dram_tensor("tri", [PPC, PPC], f16, kind="ExternalInput").ap()
    ab_d = nc.dram_tensor("AB", [4, NPX], f32, kind="ExternalOutput").ap()

    with ExitStack() as ctx:
        tc = ctx.enter_context(tile.TileContext(nc))

        singles = ctx.enter_context(tc.tile_pool(name="singles", bufs=1))
        setup = ctx.enter_context(tc.tile_pool(name="setup", bufs=1))
        work = ctx.enter_context(tc.tile_pool(name="work", bufs=3))
        lgpool = ctx.enter_context(tc.tile_pool(name="lg", bufs=2))
        vpool = ctx.enter_context(tc.tile_pool(name="vp", bufs=10))
        bstage = ctx.enter_context(tc.tile_pool(name="bst", bufs=8))
        ps_ss = ctx.enter_context(tc.tile_pool(name="pss", bufs=6, space="PSUM"))
        ps_b = ctx.enter_context(tc.tile_pool(name="psb", bufs=2, space="PSUM"))

        # ---- load inputs -------------------------------------------------
        cp_sb = singles.tile([PPC, NSEG * 4 * 2], f32)
        nc.sync.dma_start(cp_sb[:], cp_d)
        col_sb = singles.tile([PPC, 4], f32)
        nc.sync.dma_start(col_sb[:], col_d)
        lin_sb = singles.tile([PPC, W], f32)
        nc.sync.dma_start(lin_sb[:], lin_d)
        bas_sb = singles.tile([PPC, 4 * NT], f32)
        nc.sync.dma_start(bas_sb[:], bas_d)
        wng_sb = singles.tile([PPC, NSEG * 4 * 2], f32)
        nc.sync.dma_start(wng_sb[:], wng_d)
        tri_sb = singles.tile([PPC, PPC], f16)
        nc.sync.dma_start(tri_sb[:], tri_d)

        # ---- centers (negated means): a fixed 16-weight functional of the
        # control points -- ready ~3us before the full pts chain ------------
        cprod = setup.tile([PPC, NSEG * 4 * 2], f32)
        nc.vector.tensor_mul(cprod[:], cp_sb[:], wng_sb[:])
        cp3 = cprod[:].rearrange("p (s c e) -> p s c e", s=NSEG, c=4, e=2)
        neg_cx = setup.tile([PPC, 1], f32)
        nc.vector.tensor_reduce(neg_cx[:], cp3[:, :, :, 0],
                                axis=mybir.AxisListType.XY, op=ALU.add)
        neg_cy = setup.tile([PPC, 1], f32)
        nc.vector.tensor_reduce(neg_cy[:], cp3[:, :, :, 1],
                                axis=mybir.AxisListType.XY, op=ALU.add)

        # ---- bezier samples: pts[p,s,t,e] = sum_c basis[t,c]*cp[p,s,c,e] -
        cp4 = cp_sb[:].rearrange("p (s c e) -> p s c e", s=NSEG, c=4, e=2)
        bas4 = bas_sb[:].rearrange("p (c t) -> p c t", c=4, t=NT)
        prods = []
        for c in range(4):
            pc = setup.tile([PPC, NSEG, NT, 2], f32, tag=f"prod{c}")
            cpv = cp4[:, :, c, :].unsqueeze(2).broadcast_to([PPC, NSEG, NT, 2])
            bv = (bas4[:, c, :].unsqueeze(1).unsqueeze(3)
                  .broadcast_to([PPC, NSEG, NT, 2]))
            nc.vector.tensor_mul(pc[:], cpv, bv)
            prods.append(pc)
        s01 = setup.tile([PPC, NSEG, NT, 2], f32)
        nc.vector.tensor_add(s01[:], prods[0][:], prods[1][:])
        s23 = setup.tile([PPC, NSEG, NT, 2], f32)
        nc.vector.tensor_add(s23[:], prods[2][:], prods[3][:])
        pts = setup.tile([PPC, NSEG, NT, 2], f32)
        nc.vector.tensor_add(pts[:], s01[:], s23[:])

        ptx = pts[:, :, :, 0]   # [p, 4, 49]
        pty = pts[:, :, :, 1]

        # ---- avg radius -> r50 = 50 * mean ||pts - c|| -------------------
        sqx = setup.tile([PPC, NSEG, NT], f32)
        nc.scalar.activation(sqx[:], ptx, ACT.Square, bias=neg_cx[:])
        sqy = setup.tile([PPC, NSEG, NT], f32)
        nc.scalar.activation(sqy[:], pty, ACT.Square, bias=neg_cy[:])
        d2p = setup.tile([PPC, NSEG, NT], f32)
        nc.vector.tensor_add(d2p[:], sqx[:], sqy[:])
        sp = setup.tile([PPC, NSEG, NT], f32)
        rsum = setup.tile([PPC, 1], f32)
        nc.scalar.activation(sp[:], d2p[:], ACT.Sqrt, accum_out=rsum[:])
        r50 = setup.tile([PPC, 1], f32)
        nc.vector.tensor_scalar_mul(r50[:], rsum[:], INV_SOFT / NPTS)

        # ---- per-path alpha and color-diff matmul weights ----------------
        neg_alpha = setup.tile([PPC, 1], f32)
        nc.vector.tensor_scalar_mul(neg_alpha[:], col_sb[:, 3:4], -1.0)

        csh = setup.tile([PPC, 3], f32)       # c_{k-1} (0 for k=0)
        nc.vector.memset(csh[0:1, :], 0.0)
        nc.sync.dma_start(csh[1:PPC, :], col_sb[0:PPC - 1, 0:3])
        d4f = setup.tile([PPC, 4], f32)       # cols 0-2: D, col 3: e_0 (A row)
        nc.vector.tensor_sub(d4f[:, 0:3], csh[:], col_sb[:, 0:3])
        nc.vector.memset(d4f[:, 3:4], 0.0)
        nc.vector.memset(d4f[0:1, 3:4], 1.0)
        d4 = setup.tile([PPC, 4], f16)
        nc.vector.tensor_copy(d4[:], d4f[:])



        # ---- separable squared distances --------------------------------
        dx2 = singles.tile([PPC, W], f32)
        nc.scalar.activation(dx2[:], lin_sb[:], ACT.Square, bias=neg_cx[:])
        dy2 = singles.tile([PPC, W], f32)
        nc.scalar.activation(dy2[:], lin_sb[:], ACT.Square, bias=neg_cy[:])

        # ---- main loop: pairs of chunks, phase-batched to cut ACT table
        # switches (sqrt x2 | sigmoid x2 | ln x2 | exp xN per pair).
        # work pool has 3 buffers and the next pair's d2 adds are emitted
        # right after this pair's ln phase, so the DVE finishes them long
        # before the next sqrt phase (no pair-boundary stall).
        prev_act = None  # last ACT inst of previous phase, for ordering
        npairs = len(CHUNKS) // 2
        pair_rows = [(sum(CHUNKS[:2 * p]), CHUNKS[2 * p], CHUNKS[2 * p + 1])
                     for p in range(npairs)]
        tts_by_pair = {}

        def ordered_act(i):
            nonlocal prev_act
            if prev_act is not None:
                add_dep_helper(i.ins, prev_act.ins, sync=False,
                               reason="ACT table-set phase order")
            prev_act = i

        def emit_d2(p, k, prime=False):
            """DVE broadcast add d2 = dy2[rows] + dx2 for chunk k of pair p."""
            r0 = pair_rows[p][0] + (pair_rows[p][1] if k else 0)
            ch_rows = pair_rows[p][1 + k]
            t = work.tile([PPC, MAXCH_PX], f32, tag="work")
            t3 = t[:, :ch_rows * W].rearrange("p (r j) -> p r j",
                                              r=ch_rows, j=W)
            nsub = 4 if prime else 1
            rstep = -(-ch_rows // nsub)
            for rlo in range(0, ch_rows, rstep):
                rn = min(rstep, ch_rows - rlo)
                dyb = (dy2[:, r0 + rlo:r0 + rlo + rn].unsqueeze(2)
                       .broadcast_to([PPC, rn, W]))
                dxb = dx2[:].unsqueeze(1).broadcast_to([PPC, rn, W])
                nc.vector.tensor_add(t3[:, rlo:rlo + rn, :], dyb, dxb)
            tts_by_pair.setdefault(p, []).append(t)

        emit_d2(0, 0, prime=True)
        emit_d2(0, 1)

        for pair in range(npairs):
            row0, rows_a, rows_b = pair_rows[pair]
            tts = tts_by_pair.pop(pair)
            metas = [(row0, rows_a * W), (row0 + rows_a, rows_b * W)]

            for k in range(2):
                ch_px = metas[k][1]
                if pair == 0:
                    # prime the pipeline: sub-slice the first sqrt so ACT
                    # starts before the full d2 add has finished
                    step = -(-ch_px // 4)
                    for lo in range(0, ch_px, step):
                        n = min(step, ch_px - lo)
                        ordered_act(nc.scalar.activation(
                            tts[k][:, lo:lo + n], tts[k][:, lo:lo + n],
                            ACT.Sqrt))
                else:
                    ordered_act(nc.scalar.activation(
                        tts[k][:, :ch_px], tts[k][:, :ch_px], ACT.Sqrt))
            for k in range(2):
                ch_px = metas[k][1]
                ordered_act(nc.scalar.activation(
                    tts[k][:, :ch_px], tts[k][:, :ch_px],
                    ACT.Sigmoid, bias=r50[:], scale=-INV_SOFT))
            lgs = []
            for k in range(2):
                ch_px = metas[k][1]
                lg = lgpool.tile([PPC, MAXCH_PX], f16, tag="lg")
                ordered_act(nc.scalar.activation(
                    lg[:, :ch_px], tts[k][:, :ch_px],
                    ACT.Ln, bias=1.0, scale=neg_alpha[:]))
                lgs.append(lg)

            # queue the next pair's d2 now: one work buffer is already free
            # and the second frees after this pair's first ln
            if pair + 1 < npairs:
                emit_d2(pair + 1, 0)
                emit_d2(pair + 1, 1)

            for k in range(2):
                crow0, ch_px = metas[k]
                lg = lgs[k]
                ngrp = (ch_px + XBLK - 1) // XBLK
                for g in range(ngrp):
                    lo = g * XBLK
                    gw = min(XBLK, ch_px - lo)
                    px0 = crow0 * W + lo

                    ss = ps_ss.tile([PPC, XBLK], f32, tag="ss")
                    nc.tensor.matmul(ss[:, :gw], tri_sb[:],
                                     lg[:, lo:lo + gw],
                                     start=True, stop=True)
                    v = vpool.tile([PPC, XBLK], f16, tag="v")
                    ordered_act(nc.scalar.activation(v[:, :gw], ss[:, :gw],
                                                     ACT.Exp))

                    bp = ps_b.tile([4, XBLK], f32, tag="bp")
                    nc.tensor.matmul(bp[:, :gw], d4[:], v[:, :gw],
                                     start=True, stop=True)
                    bs = bstage.tile([4, XBLK], f32, tag="bs")
                    nc.vector.tensor_copy(bs[:, :gw], bp[:, :gw])
                    nc.sync.dma_start(ab_d[:, px0:px0 + gw], bs[:, :gw])

    nc.compile()
    return nc


def _get_nc():
    if "nc" not in _compiled:
        _compiled["nc"] = _build_nc()
    return _compiled["nc"]


def _bezier_basis():
    t = np.linspace(0.0, 1.0, NSAMP, dtype=np.float32)[:-1]
    mt = 1.0 - t
    return np.stack([mt ** 3, 3.0 * mt ** 2 * t, 3.0 * mt * t ** 2, t ** 3],
                    axis=-1).astype(np.float32)  # (49, 4)


def _run_on_device(cp, col):
    """Compile (cached) + run the SPMD kernel; returns list of AB arrays."""
    global last_results
    from concourse.bass_utils import run_bass_kernel_spmd

    basis = _bezier_basis()                       # (49, 4)
    bas_in = np.broadcast_to(basis.T.reshape(1, 4 * NT),
                             (PPC, 4 * NT)).copy()  # rows: c-major
    lin = np.linspace(0.0, 1.0, W, dtype=np.float32)
    lin_in = np.broadcast_to(lin, (PPC, W)).copy()
    u = -(basis.sum(axis=0) / NPTS).astype(np.float32)       # (4,) per c
    wneg = np.broadcast_to(np.repeat(u, 2)[None, None, :],
                           (PPC, NSEG, 8)).reshape(PPC, 32).copy()
    q = np.arange(PPC)
    tri = (q[:, None] >= q[None, :]).astype(np.float16)  # tri[q,k] = q>=k

    nc = _get_nc()
    in_maps = []
    for s in range(NCORES):
        sl = slice(s * PPC, (s + 1) * PPC)
        in_maps.append({
            "cp": cp[sl].reshape(PPC, NSEG * 4 * 2).copy(),
            "col": col[sl].copy(),
            "lin_t": lin_in,
            "basis": bas_in,
            "wneg": wneg,
            "tri": tri,
        })

    res = run_bass_kernel_spmd(nc, in_maps, core_ids=list(range(NCORES)))
    last_results = res
    return [res.results[s]["AB"] for s in range(NCORES)]


def _subproc_main(in_path, out_path):
    data = np.load(in_path)
    abs_ = _run_on_device(data["cp"], data["col"])
    np.savez(out_path, **{f"ab{s}": ab for s, ab in enumerate(abs_)})


def _run_with_recovery(cp, col):
    """The NeuronCore runtime occasionally reports a transient
    NRT_EXEC_UNIT_UNRECOVERABLE on a cold run, which poisons the whole PJRT
    client in this process. A fresh process reliably recovers, so fall back
    to re-running in a subprocess."""
    import os
    import subprocess
    import sys as _sys
    import tempfile

    try:
        return _run_on_device(cp, col)
    except Exception:
        pass
    last_exc = None
    for _ in range(3):
        tmp = tempfile.mkdtemp()
        in_path = os.path.join(tmp, "in.npz")
        out_path = os.path.join(tmp, "out.npz")
        np.savez(in_path, cp=cp, col=col)
        code = ("import sys; sys.path.insert(0, %r); import kernel; "
                "kernel._subproc_main(%r, %r)"
                % (os.path.dirname(os.path.abspath(__file__)),
                   in_path, out_path))
        try:
            subprocess.run([_sys.executable, "-c", code], check=True,
                           timeout=1200)
            data = np.load(out_path)
            return [data[f"ab{s}"] for s in range(NCORES)]
        except Exception as e:  # noqa: PERF203
            last_exc = e
    raise last_exc


def kernel(paths_control_points, colors):
    cp = np.ascontiguousarray(paths_control_points, dtype=np.float32)
    col = np.ascontiguousarray(colors, dtype=np.float32)

    abs_ = _run_with_recovery(cp, col)

    canvas = np.ones((3, H, W), dtype=np.float32)
    for s in range(NCORES):
        ab = abs_[s]
        a = ab[3].reshape(H, W)
        b = ab[0:3].reshape(3, H, W)
        c_last = col[s * PPC + PPC - 1, 0:3]
        canvas = canvas * a[None] + b + c_last[:, None, None]
    return canvas.astype(np.float32)
